# revision 51
# baseline (speedup 1.0000x reference)
"""
Trainium2 Bass kernel for nn_CausalMatrixGameTransformerBlock (streaming-window
attention), v2.

Math (static shapes from the spec): B=1, S=1920 new tokens, N=12 heads, D=128,
CACHE=6720, f=2, h=24, w=40, current_start=global_end=local_end=5760. With
those ints the reference reduces to, per head:
    K = concat(cache_k[:, 1920:5760], rope(k))   # [5760, 128]
    V = concat(cache_v[:, 1920:5760], v)
    out = softmax(rope(q) K^T / sqrt(128)) V     # dense over 5760 keys

Division of labor (the device does all O(S*W) work; host does O(S) prep/post):
  host   : RoPE of q/k, layout transposes, final out = po / denom divide.
  device : QK^T scores (PE, f32r), exp (ACT, bf16 out), PV (PE, bf16),
           denominator partials (DVE bf16 running sum over the 45 kk-tiles).

Sharding: 24 units of (head, 960-query half); core c owns units 3c..3c+2 --
always one head with both halves plus one head with a single half, so each
core loads exactly 2 heads' K/V.

Per-core device schedule (flat software pipeline; ACT is the critical engine
at ~(1440+222) cycles per 3-tile exp group, so everything else is arranged to
keep it saturated):
  for kv in {A (2 q-blocks), B (1 q-block)}: for each 480-wide q chunk:
    for g in 0..14 (3 kk-tiles per group):
      ps[128,3,512]   = 3x matmul KT_t^T @ q_chunk        (PE, f32r, 480 cols)
      ex[128,3,480]   = exp(ps * 1/sqrt(128))             (ACT, one FD=1440
                        instruction per group -- PSUM in, bf16 SBUF out;
                        group 0's first tile goes to DVE as a factored
                        quartic (q4(u))^16 instead, shedding ACT work)
      acc[128,1440]  += ex                                (DVE bf16 2x)
      po[128,480]    += 3x matmul V_t^T @ ex[:,i,:]       (PE, bf16, emitted
                        TWO groups late so the in-order PE never has a PV
                        (gated on exp) ahead of the QK feeding the next exp)
    DMA out po (fp32) + the unfolded acc (bf16); host does the 128-partition
    denominator sum and the final divide.
Other details: PE p-state warm-up matmuls at t~0; kT/vT DMA'd in pieces sized
so each lands just before its first QK/PV group; all 8 PSUM banks used
(2 x 3-bank score buffers double-buffered + 2 x 1-bank po accumulators).
"""

import math
import numpy as np

N_CORES = 8
S = 1920
NHEADS = 12
D = 128
WIN = 5760           # attention window (keys)
KTILES = WIN // 128  # 45
UQ = 960             # queries per (head, half) unit
QCHUNK = 480
NGROUPS = KTILES // 3  # 15 groups of 3 kk-tiles per q chunk
POLYG = 0            # group whose first tile is exp-d on DVE instead of ACT

_PROG = None


def _rope_tables():
    """cos/sin angle tables [1920, 64] exactly as the reference builds them."""
    def rope_angles(max_len, dim, theta=10000.0):
        inv = 1.0 / (theta ** (np.arange(0, dim, 2, dtype=np.float64) / dim))
        return np.outer(np.arange(max_len, dtype=np.float64), inv)

    d = D
    freqs = np.concatenate([
        rope_angles(1024, d - 4 * (d // 6)),
        rope_angles(1024, 2 * (d // 6)),
        rope_angles(1024, 2 * (d // 6)),
    ], axis=1).astype(np.float32)          # [1024, 64]

    f, h, w = 2, 24, 40
    start_frame = 6                         # current_start // (h*w)
    c = d // 2
    s0, s1 = c - 2 * (c // 3), c // 3       # 22, 21
    ang = np.concatenate([
        np.broadcast_to(freqs[start_frame:start_frame + f, :s0][:, None, None, :], (f, h, w, s0)),
        np.broadcast_to(freqs[:h, s0:s0 + s1][None, :, None, :], (f, h, w, s1)),
        np.broadcast_to(freqs[:w, s0 + s1:][None, None, :, :], (f, h, w, s1)),
    ], axis=-1).reshape(S, c)
    return np.cos(ang).astype(np.float32), np.sin(ang).astype(np.float32)


def _rope_host(x, cos, sin):
    """x: [S, N, D]; complex rotation on even/odd pairs (reference layout)."""
    x0, x1 = x[..., 0::2], x[..., 1::2]
    c, s = cos[:, None, :], sin[:, None, :]
    out = np.empty_like(x)
    out[..., 0::2] = x0 * c - x1 * s
    out[..., 1::2] = x0 * s + x1 * c
    return out


def _units_for_core(c):
    return [((u // 2), (u % 2)) for u in range(3 * c, 3 * c + 3)]


def _core_heads(c):
    """(double_head, its halves order-matched to qT[0:2]), (single_head, half)."""
    units = _units_for_core(c)
    from collections import Counter
    cnt = Counter(n for n, _ in units)
    dbl = next(n for n, k in cnt.items() if k == 2)
    sgl = next(n for n, k in cnt.items() if k == 1)
    dbl_halves = [h for n, h in units if n == dbl]
    sgl_half = next(h for n, h in units if n == sgl)
    return dbl, dbl_halves, sgl, sgl_half


def _build_program():
    from contextlib import ExitStack
    from concourse import bacc
    import concourse.tile as tile
    import concourse.mybir as mybir

    from concourse.alu_op_type import AluOpType

    F32 = mybir.dt.float32
    F32R = mybir.dt.float32r
    BF16 = mybir.dt.bfloat16
    EXP = mybir.ActivationFunctionType.Exp
    SCALE = 1.0 / math.sqrt(float(D))

    # Quartic-factored exp for the one tile per chunk that DVE computes:
    # exp(x*SCALE) ~= (K*(x^2+A1*x+B1)*(x^2+A2*x+B2))^16, max rel err 8e-4
    # over |x*SCALE| <= 5.44. Constants from factoring 1+u+u^2/2+u^3/6+u^4/24
    # (u = x*SCALE/16) into (1/24)(u^2+a1 u+b1)(u^2+a2 u+b2).
    _t = SCALE / 16.0
    A1, B1 = 0.5411115378645888 / _t, 6.347102755177203 / (_t * _t)
    A2, B2 = 3.45888846213541 / _t, 3.7812527897746233 / (_t * _t)
    KQ = _t ** 4 / 24.0

    nc = bacc.Bacc("TRN2", target_bir_lowering=False, debug=False,
                   enable_asserts=False, num_devices=N_CORES)

    qTd = nc.dram_tensor("qT", [3, 128, UQ], F32, kind="ExternalInput").ap()
    kTd = nc.dram_tensor("kT", [2, 128, WIN], F32, kind="ExternalInput").ap()
    vTd = nc.dram_tensor("vT", [2, 128, WIN], BF16, kind="ExternalInput").ap()
    outp = nc.dram_tensor("outp", [3, 2, 128, QCHUNK], F32, kind="ExternalOutput").ap()
    dens = nc.dram_tensor("dens", [3, 2, 128, 3 * QCHUNK], BF16, kind="ExternalOutput").ap()
    dens2 = nc.dram_tensor("dens2", [128, 3 * QCHUNK], BF16, kind="ExternalOutput").ap()
    densr = nc.dram_tensor("densr", [3, 2, 2, 128, 3 * QCHUNK], BF16, kind="ExternalOutput").ap()

    # kT DMA split: small first pieces so the first QK groups start early.
    KCUTS = [0, 384, 1536, 3456, WIN]
    VCUTS = [0, 768, 2944, WIN]

    with ExitStack() as ctx:
        tc = ctx.enter_context(tile.TileContext(nc))
        kpool = ctx.enter_context(tc.tile_pool(name="kp", bufs=2))
        vpool = ctx.enter_context(tc.tile_pool(name="vp", bufs=2))
        qpool = ctx.enter_context(tc.tile_pool(name="qp", bufs=6))
        expool = ctx.enter_context(tc.tile_pool(name="exp", bufs=6))
        accp = ctx.enter_context(tc.tile_pool(name="acp", bufs=2))
        outsb = ctx.enter_context(tc.tile_pool(name="obp", bufs=2))
        polyp = ctx.enter_context(tc.tile_pool(name="pyp", bufs=4))
        pss = ctx.enter_context(tc.tile_pool(name="pss", bufs=2, space="PSUM"))
        pop = ctx.enter_context(tc.tile_pool(name="pop", bufs=2, space="PSUM"))

        def load_q(qb, c):
            qT = qpool.tile([128, QCHUNK], F32R, name="qT")
            nc.sync.dma_start(out=qT,
                              in_=qTd[qb, :, c * QCHUNK:(c + 1) * QCHUNK].bitcast(F32R))
            return qT

        def load_kv(kv, first):
            KT = kpool.tile([128, WIN], F32R, name="KT")
            VT = vpool.tile([128, WIN], BF16, name="VT")

            def kpiece(i):
                a, b = KCUTS[i], KCUTS[i + 1]
                nc.sync.dma_start(out=KT[:, a:b], in_=kTd[kv, :, a:b].bitcast(F32R))

            def vpiece(i):
                a, b = VCUTS[i], VCUTS[i + 1]
                nc.sync.dma_start(out=VT[:, a:b], in_=vTd[kv, :, a:b])

            qT0 = None
            kpiece(0)
            if first:
                qT0 = load_q(0, 0)
            kpiece(1)
            vpiece(0)
            kpiece(2)
            vpiece(1)
            kpiece(3)
            vpiece(2)
            return KT, VT, qT0

        # PE p-state warm-up: ~3.8us of junk matmuls starting at t~0 so the
        # first real QK matmuls dispatch at the full 2.4 GHz clock. Inputs are
        # uninitialized SBUF; the target PSUM region is overwritten by the
        # first start=True QK matmul that later lands in the same bank.
        jk = qpool.tile([128, 128], F32, name="jk")
        nc.gpsimd.memset(jk, 0.0)
        warm = pss.tile([128, 3, 512], F32, name="ps")
        for _ in range(8):
            nc.tensor.matmul(out=warm[:, 0, 0:128], lhsT=jk, rhs=jk,
                             start=True, stop=True)

        # (kv index, q-block index, output unit slot)
        SCHED = [(0, 0), (0, 1), (1, 2)]

        KT, VT, q00 = load_kv(0, first=True)
        kvtiles = {0: (KT, VT)}
        qtiles = {(0, 0): q00}
        # 2-deep PV pipeline: drain PV(g) only after QK(g+2) is emitted, so the
        # in-order PE never has a PV (gated on exp(g)) ahead of the QK group
        # that feeds the next exp.
        pend = []   # FIFO of (ex, VT, po, g, epilogue | None, first)
        stash = []  # deferred final acc add of the previous chunk

        def drain(p):
            ex_, VT_, po_, g_, epi_, first_ = p
            for i in range(3):
                t = 3 * g_ + i
                nc.tensor.matmul(out=po_, lhsT=VT_[:, t * 128:(t + 1) * 128],
                                 rhs=ex_[:, i, :], start=(first_ and i == 0),
                                 stop=(t == KTILES - 1))
            if epi_ is not None:
                epi_()

        PW = 2 * QCHUNK

        def poly_exp(xps, out, after_copy=None):
            """out = exp(x*SCALE) elementwise on DVE (xps: [128,960] fp32 PSUM)."""
            x = polyp.tile([128, PW], F32, name="px")
            nc.vector.tensor_copy(out=x, in_=xps)
            if after_copy is not None:
                # squeeze the previous chunk's final acc add in right after the
                # ps-freeing copy so it doesn't delay the next QK group
                after_copy()
            p1 = polyp.tile([128, PW], F32, name="p1")
            nc.vector.scalar_tensor_tensor(p1, x, A1, x, AluOpType.add, AluOpType.mult)
            p2 = polyp.tile([128, PW], F32, name="p2")
            nc.vector.scalar_tensor_tensor(p2, x, A2, x, AluOpType.add, AluOpType.mult)
            q1 = polyp.tile([128, PW], F32, name="q1")
            nc.vector.tensor_scalar(q1, p1, B1, KQ, AluOpType.add, AluOpType.mult)
            y = polyp.tile([128, PW], F32, name="y0")
            nc.vector.scalar_tensor_tensor(y, p2, B2, q1, AluOpType.add, AluOpType.mult)
            for r in range(4):
                yn = out if r == 3 else polyp.tile([128, PW], F32, name="yn")
                nc.vector.tensor_mul(yn, y, y)
                y = yn

        for si, (kv, qb) in enumerate(SCHED):
            KT, VT = kvtiles[kv]
            for c in range(2):
                qs = qtiles[(qb, c)]
                po = pop.tile([128, QCHUNK], F32, name="po")
                acc = accp.tile([128, 3 * QCHUNK], BF16, name="acc")
                for g in range(NGROUPS):
                    ps = pss.tile([128, 3, 512], F32, name="ps")
                    for i in range(3):
                        t = 3 * g + i
                        nc.tensor.matmul(out=ps[:, i, 0:QCHUNK],
                                         lhsT=KT[:, t * 128:(t + 1) * 128],
                                         rhs=qs, start=True, stop=True)
                    if si == 0 and c == 0 and g == 0:
                        # prefetch the second head's K/V + remaining q blocks
                        qtiles[(0, 1)] = load_q(0, 1)
                        kvtiles[1] = load_kv(1, first=False)[:2]
                        for pqb, pc in ((1, 0), (1, 1), (2, 0), (2, 1)):
                            qtiles[(pqb, pc)] = load_q(pqb, pc)
                    if g == POLYG:
                        ex = expool.tile([128, 3, QCHUNK], BF16, name="ex0", bufs=2)
                        # tile 0 of this group is computed by DVE (poly); ACT
                        # does the other two, shedding one tile per chunk
                        nc.scalar.activation(out=ex[:, 2:3, :],
                                             in_=ps[:, 2:3, 0:QCHUNK],
                                             func=EXP, scale=SCALE)
                        pov = ex[:, 0:2, :].rearrange("p a b -> p (a b)")
                        poly_exp(ps[:, 0:2, 0:QCHUNK], pov,
                                 after_copy=stash.pop() if stash else None)
                    else:
                        ex = expool.tile([128, 3, QCHUNK], BF16, name="ex")
                        nc.scalar.activation(out=ex, in_=ps[:, :, 0:QCHUNK],
                                             func=EXP, scale=SCALE)
                    exv = ex.rearrange("p a b -> p (a b)")
                    if g == POLYG:
                        gpex, gpexv = ex, exv
                    elif g == (1 if POLYG == 0 else 0):
                        firstexv = exv
                    elif g == 2:
                        nc.vector.tensor_add(acc, firstexv, exv)
                    elif g in (12, 13):
                        # raw-ship: host folds these into the denominator
                        nc.sync.dma_start(out=densr[si, c, g - 12], in_=exv)
                    elif g == NGROUPS - 1:
                        if (si, c) == (len(SCHED) - 1, 1):
                            # last chunk: ship this group raw (host adds it);
                            # the tail then never waits on a final DVE add
                            last_exv = exv
                        else:
                            # stash the final add; emitted after the next
                            # chunk's ps-freeing poly copy
                            def _fin(acc=acc, exv=exv):
                                nc.vector.tensor_add(acc, acc, exv)
                            stash.append(_fin)
                    else:
                        nc.vector.tensor_add(acc, acc, exv)
                        if g == 11:
                            # poly group's contribution, ready by now
                            nc.vector.tensor_add(acc, acc, gpexv)
                    if g == 10:
                        # deferred PV for the poly group (waits on the chain)
                        drain((gpex, VT, po, POLYG, None, False))
                    if len(pend) >= 2:
                        drain(pend.pop(0))
                    epi = None
                    if g == NGROUPS - 1:
                        # chunk epilogue, emitted only after this group's PV
                        # drains so the po copy orders after the last matmul
                        def epi(si=si, c=c, acc=acc, po=po, ex=ex):
                            nc.sync.dma_start(out=dens[si, c], in_=acc)
                            if (si, c) == (len(SCHED) - 1, 1):
                                nc.sync.dma_start(
                                    out=dens2, in_=ex.rearrange("p a b -> p (a b)"))
                            osb = outsb.tile([128, QCHUNK], F32, name="osb")
                            nc.vector.tensor_copy(out=osb, in_=po)
                            nc.sync.dma_start(out=outp[si, c], in_=osb)
                    if g != POLYG:
                        pend.append((ex, VT, po, g, epi,
                                     g == (1 if POLYG == 0 else 0)))
        if stash:
            stash.pop()()
        for p in pend:
            drain(p)

    nc.compile()
    return nc


def _get_program():
    global _PROG
    if _PROG is None:
        _PROG = _build_program()
    return _PROG


def _host_prep(q, k, v, cache_k, cache_v):
    """Build the 8 per-core input maps (rope + layout on host)."""
    import ml_dtypes
    cos, sin = _rope_tables()
    q = np.asarray(q, np.float32)[0]
    k = np.asarray(k, np.float32)[0]
    v = np.asarray(v, np.float32)[0]
    ck = np.asarray(cache_k, np.float32)[0, 1920:5760]
    cv = np.asarray(cache_v, np.float32)[0, 1920:5760]
    rq = _rope_host(q, cos, sin)                     # [1920, 12, 128]
    rk = _rope_host(k, cos, sin)
    Kfull = np.concatenate([ck, rk], axis=0)         # [5760, 12, 128]
    Vfull = np.concatenate([cv, v], axis=0)

    in_maps = []
    for cidx in range(N_CORES):
        dbl, dbl_halves, sgl, sgl_half = _core_heads(cidx)
        qT = np.stack([
            np.ascontiguousarray(rq[dbl_halves[0] * UQ:(dbl_halves[0] + 1) * UQ, dbl, :].T),
            np.ascontiguousarray(rq[dbl_halves[1] * UQ:(dbl_halves[1] + 1) * UQ, dbl, :].T),
            np.ascontiguousarray(rq[sgl_half * UQ:(sgl_half + 1) * UQ, sgl, :].T),
        ])
        kT = np.stack([np.ascontiguousarray(Kfull[:, n, :].T) for n in (dbl, sgl)])
        vT = np.stack([
            np.ascontiguousarray(
                Vfull[:, n, :].reshape(KTILES, 128, 128).transpose(1, 0, 2).reshape(128, WIN))
            for n in (dbl, sgl)
        ]).astype(ml_dtypes.bfloat16)
        in_maps.append({"qT": qT, "kT": kT, "vT": vT})
    return in_maps


def _gather(results):
    out = np.empty((1, S, NHEADS, D), np.float32)
    for cidx in range(N_CORES):
        dbl, dbl_halves, sgl, sgl_half = _core_heads(cidx)
        slots = [(dbl, dbl_halves[0]), (dbl, dbl_halves[1]), (sgl, sgl_half)]
        po = np.asarray(results[cidx]["outp"], np.float32)    # [3, 2, 128, 480]
        de = np.asarray(results[cidx]["dens"], np.float32)    # [3, 2, 128, 1440]
        dr = np.asarray(results[cidx]["densr"], np.float32)   # [3, 2, 2, 128, 1440]
        for s_i, (n, half) in enumerate(slots):
            for c in range(2):
                denom = de[s_i, c].reshape(128, 3, QCHUNK).sum(axis=(0, 1))
                denom = denom + dr[s_i, c].reshape(2 * 128, 3, QCHUNK).sum(axis=(0, 1))
                if (s_i, c) == (2, 1):
                    denom = denom + np.asarray(results[cidx]["dens2"], np.float32)\
                        .reshape(128, 3, QCHUNK).sum(axis=(0, 1))
                o = po[s_i, c] / denom[None, :]                # [128, 480]
                q0 = half * UQ + c * QCHUNK
                out[0, q0:q0 + QCHUNK, n, :] = o.T
    return out


def kernel(q, k, v, cache_k, cache_v, f=2, h=24, w=40,
           current_start=5760, global_end=5760, local_end=5760, **_extra):
    from concourse.bass_utils import run_bass_kernel_spmd

    nc = _get_program()
    in_maps = _host_prep(q, k, v, cache_k, cache_v)
    res = run_bass_kernel_spmd(nc, in_maps, list(range(N_CORES)))
    return _gather(res.results)


# revision 52
# speedup vs baseline: 1.0415x; 1.0415x over previous
"""
Trainium2 Bass kernel for nn_CausalMatrixGameTransformerBlock (streaming-window
attention), v2.

Math (static shapes from the spec): B=1, S=1920 new tokens, N=12 heads, D=128,
CACHE=6720, f=2, h=24, w=40, current_start=global_end=local_end=5760. With
those ints the reference reduces to, per head:
    K = concat(cache_k[:, 1920:5760], rope(k))   # [5760, 128]
    V = concat(cache_v[:, 1920:5760], v)
    out = softmax(rope(q) K^T / sqrt(128)) V     # dense over 5760 keys

Division of labor (the device does all O(S*W) work; host does O(S) prep/post):
  host   : RoPE of q/k, layout transposes, final out = po / denom divide.
  device : QK^T scores (PE, f32r), exp (ACT, bf16 out), PV (PE, bf16),
           denominator partials (DVE bf16 running sum over the 45 kk-tiles).

Sharding: 24 units of (head, 960-query half); core c owns units 3c..3c+2 --
always one head with both halves plus one head with a single half, so each
core loads exactly 2 heads' K/V.

Per-core device schedule (flat software pipeline; ACT is the critical engine
at ~(1440+222) cycles per 3-tile exp group, so everything else is arranged to
keep it saturated):
  for kv in {A (2 q-blocks), B (1 q-block)}: for each 480-wide q chunk:
    for g in 0..14 (3 kk-tiles per group):
      ps[128,3,512]   = 3x matmul KT_t^T @ q_chunk        (PE, f32r, 480 cols)
      ex[128,3,480]   = exp(ps * 1/sqrt(128))             (ACT, one FD=1440
                        instruction per group -- PSUM in, bf16 SBUF out;
                        group 0's first tile goes to DVE as a factored
                        quartic (q4(u))^16 instead, shedding ACT work)
      acc[128,1440]  += ex                                (DVE bf16 2x)
      po[128,480]    += 3x matmul V_t^T @ ex[:,i,:]       (PE, bf16, emitted
                        TWO groups late so the in-order PE never has a PV
                        (gated on exp) ahead of the QK feeding the next exp)
    DMA out po (fp32) + the unfolded acc (bf16); host does the 128-partition
    denominator sum and the final divide.
Other details: PE p-state warm-up matmuls at t~0; kT/vT DMA'd in pieces sized
so each lands just before its first QK/PV group; all 8 PSUM banks used
(2 x 3-bank score buffers double-buffered + 2 x 1-bank po accumulators).
"""

import math
import numpy as np

N_CORES = 8
S = 1920
NHEADS = 12
D = 128
WIN = 5760           # attention window (keys)
KTILES = WIN // 128  # 45
UQ = 960             # queries per (head, half) unit
QCHUNK = 480
NGROUPS = KTILES // 3  # 15 groups of 3 kk-tiles per q chunk
POLYG = 0            # group whose first tile is exp-d on DVE instead of ACT

_PROG = None


def _rope_tables():
    """cos/sin angle tables [1920, 64] exactly as the reference builds them."""
    def rope_angles(max_len, dim, theta=10000.0):
        inv = 1.0 / (theta ** (np.arange(0, dim, 2, dtype=np.float64) / dim))
        return np.outer(np.arange(max_len, dtype=np.float64), inv)

    d = D
    freqs = np.concatenate([
        rope_angles(1024, d - 4 * (d // 6)),
        rope_angles(1024, 2 * (d // 6)),
        rope_angles(1024, 2 * (d // 6)),
    ], axis=1).astype(np.float32)          # [1024, 64]

    f, h, w = 2, 24, 40
    start_frame = 6                         # current_start // (h*w)
    c = d // 2
    s0, s1 = c - 2 * (c // 3), c // 3       # 22, 21
    ang = np.concatenate([
        np.broadcast_to(freqs[start_frame:start_frame + f, :s0][:, None, None, :], (f, h, w, s0)),
        np.broadcast_to(freqs[:h, s0:s0 + s1][None, :, None, :], (f, h, w, s1)),
        np.broadcast_to(freqs[:w, s0 + s1:][None, None, :, :], (f, h, w, s1)),
    ], axis=-1).reshape(S, c)
    return np.cos(ang).astype(np.float32), np.sin(ang).astype(np.float32)


def _rope_host(x, cos, sin):
    """x: [S, N, D]; complex rotation on even/odd pairs (reference layout)."""
    x0, x1 = x[..., 0::2], x[..., 1::2]
    c, s = cos[:, None, :], sin[:, None, :]
    out = np.empty_like(x)
    out[..., 0::2] = x0 * c - x1 * s
    out[..., 1::2] = x0 * s + x1 * c
    return out


def _units_for_core(c):
    return [((u // 2), (u % 2)) for u in range(3 * c, 3 * c + 3)]


def _core_heads(c):
    """(double_head, its halves order-matched to qT[0:2]), (single_head, half)."""
    units = _units_for_core(c)
    from collections import Counter
    cnt = Counter(n for n, _ in units)
    dbl = next(n for n, k in cnt.items() if k == 2)
    sgl = next(n for n, k in cnt.items() if k == 1)
    dbl_halves = [h for n, h in units if n == dbl]
    sgl_half = next(h for n, h in units if n == sgl)
    return dbl, dbl_halves, sgl, sgl_half


def _build_program():
    from contextlib import ExitStack
    from concourse import bacc
    import concourse.tile as tile
    import concourse.mybir as mybir

    from concourse.alu_op_type import AluOpType

    F32 = mybir.dt.float32
    F32R = mybir.dt.float32r
    BF16 = mybir.dt.bfloat16
    EXP = mybir.ActivationFunctionType.Exp
    SCALE = 1.0 / math.sqrt(float(D))

    # Quartic-factored exp for the one tile per chunk that DVE computes:
    # exp(x*SCALE) ~= (K*(x^2+A1*x+B1)*(x^2+A2*x+B2))^16, max rel err 8e-4
    # over |x*SCALE| <= 5.44. Constants from factoring 1+u+u^2/2+u^3/6+u^4/24
    # (u = x*SCALE/16) into (1/24)(u^2+a1 u+b1)(u^2+a2 u+b2).
    _t = SCALE / 16.0
    A1, B1 = 0.5411115378645888 / _t, 6.347102755177203 / (_t * _t)
    A2, B2 = 3.45888846213541 / _t, 3.7812527897746233 / (_t * _t)
    KQ = _t ** 4 / 24.0

    nc = bacc.Bacc("TRN2", target_bir_lowering=False, debug=False,
                   enable_asserts=False, num_devices=N_CORES)

    qTd = nc.dram_tensor("qT", [3, 128, UQ], F32, kind="ExternalInput").ap()
    kTd = nc.dram_tensor("kT", [2, 128, WIN], F32, kind="ExternalInput").ap()
    vTd = nc.dram_tensor("vT", [2, 128, WIN], BF16, kind="ExternalInput").ap()
    outp = nc.dram_tensor("outp", [3, 2, 128, QCHUNK], F32, kind="ExternalOutput").ap()
    dens = nc.dram_tensor("dens", [3, 2, 128, 3 * QCHUNK], BF16, kind="ExternalOutput").ap()
    dens2 = nc.dram_tensor("dens2", [128, 3 * QCHUNK], BF16, kind="ExternalOutput").ap()

    # kT DMA split: small first pieces so the first QK groups start early.
    KCUTS = [0, 384, 1536, 3456, WIN]
    VCUTS = [0, 768, 2944, WIN]

    with ExitStack() as ctx:
        tc = ctx.enter_context(tile.TileContext(nc))
        kpool = ctx.enter_context(tc.tile_pool(name="kp", bufs=2))
        vpool = ctx.enter_context(tc.tile_pool(name="vp", bufs=2))
        qpool = ctx.enter_context(tc.tile_pool(name="qp", bufs=6))
        expool = ctx.enter_context(tc.tile_pool(name="exp", bufs=6))
        accp = ctx.enter_context(tc.tile_pool(name="acp", bufs=2))
        outsb = ctx.enter_context(tc.tile_pool(name="obp", bufs=2))
        polyp = ctx.enter_context(tc.tile_pool(name="pyp", bufs=4))
        pss = ctx.enter_context(tc.tile_pool(name="pss", bufs=2, space="PSUM"))
        pop = ctx.enter_context(tc.tile_pool(name="pop", bufs=2, space="PSUM"))

        def load_q(qb, c):
            qT = qpool.tile([128, QCHUNK], F32R, name="qT")
            nc.sync.dma_start(out=qT,
                              in_=qTd[qb, :, c * QCHUNK:(c + 1) * QCHUNK].bitcast(F32R))
            return qT

        def load_kv(kv, first):
            KT = kpool.tile([128, WIN], F32R, name="KT")
            VT = vpool.tile([128, WIN], BF16, name="VT")

            def kpiece(i):
                a, b = KCUTS[i], KCUTS[i + 1]
                nc.sync.dma_start(out=KT[:, a:b], in_=kTd[kv, :, a:b].bitcast(F32R))

            def vpiece(i):
                a, b = VCUTS[i], VCUTS[i + 1]
                nc.sync.dma_start(out=VT[:, a:b], in_=vTd[kv, :, a:b])

            qT0 = None
            kpiece(0)
            if first:
                qT0 = load_q(0, 0)
            kpiece(1)
            vpiece(0)
            kpiece(2)
            vpiece(1)
            kpiece(3)
            vpiece(2)
            return KT, VT, qT0

        # PE p-state warm-up: ~3.8us of junk matmuls starting at t~0 so the
        # first real QK matmuls dispatch at the full 2.4 GHz clock. Inputs are
        # uninitialized SBUF; the target PSUM region is overwritten by the
        # first start=True QK matmul that later lands in the same bank.
        jk = qpool.tile([128, 128], F32, name="jk")
        nc.gpsimd.memset(jk, 0.0)
        warm = pss.tile([128, 3, 512], F32, name="ps")
        for _ in range(8):
            nc.tensor.matmul(out=warm[:, 0, 0:128], lhsT=jk, rhs=jk,
                             start=True, stop=True)

        # (kv index, q-block index, output unit slot)
        SCHED = [(0, 0), (0, 1), (1, 2)]

        KT, VT, q00 = load_kv(0, first=True)
        kvtiles = {0: (KT, VT)}
        qtiles = {(0, 0): q00}
        # 2-deep PV pipeline: drain PV(g) only after QK(g+2) is emitted, so the
        # in-order PE never has a PV (gated on exp(g)) ahead of the QK group
        # that feeds the next exp.
        pend = []   # FIFO of (ex, VT, po, g, epilogue | None, first)
        stash = []  # deferred final acc add of the previous chunk

        def drain(p):
            ex_, VT_, po_, g_, epi_, first_ = p
            for i in range(3):
                t = 3 * g_ + i
                nc.tensor.matmul(out=po_, lhsT=VT_[:, t * 128:(t + 1) * 128],
                                 rhs=ex_[:, i, :], start=(first_ and i == 0),
                                 stop=(t == KTILES - 1))
            if epi_ is not None:
                epi_()

        def poly_exp(xps, out, after_copy=None):
            """out = exp(x*SCALE) elementwise on DVE (xps: [128,480] fp32 PSUM)."""
            x = polyp.tile([128, QCHUNK], F32, name="px")
            nc.vector.tensor_copy(out=x, in_=xps)
            if after_copy is not None:
                # squeeze the previous chunk's final acc add in right after the
                # ps-freeing copy so it doesn't delay the next QK group
                after_copy()
            p1 = polyp.tile([128, QCHUNK], F32, name="p1")
            nc.vector.scalar_tensor_tensor(p1, x, A1, x, AluOpType.add, AluOpType.mult)
            p2 = polyp.tile([128, QCHUNK], F32, name="p2")
            nc.vector.scalar_tensor_tensor(p2, x, A2, x, AluOpType.add, AluOpType.mult)
            q1 = polyp.tile([128, QCHUNK], F32, name="q1")
            nc.vector.tensor_scalar(q1, p1, B1, KQ, AluOpType.add, AluOpType.mult)
            y = polyp.tile([128, QCHUNK], F32, name="y0")
            nc.vector.scalar_tensor_tensor(y, p2, B2, q1, AluOpType.add, AluOpType.mult)
            for r in range(4):
                yn = out if r == 3 else polyp.tile([128, QCHUNK], F32, name="yn")
                nc.vector.tensor_mul(yn, y, y)
                y = yn

        for si, (kv, qb) in enumerate(SCHED):
            KT, VT = kvtiles[kv]
            for c in range(2):
                qs = qtiles[(qb, c)]
                po = pop.tile([128, QCHUNK], F32, name="po")
                acc = accp.tile([128, 3 * QCHUNK], BF16, name="acc")
                for g in range(NGROUPS):
                    ps = pss.tile([128, 3, 512], F32, name="ps")
                    for i in range(3):
                        t = 3 * g + i
                        nc.tensor.matmul(out=ps[:, i, 0:QCHUNK],
                                         lhsT=KT[:, t * 128:(t + 1) * 128],
                                         rhs=qs, start=True, stop=True)
                    if si == 0 and c == 0 and g == 0:
                        # prefetch the second head's K/V + remaining q blocks
                        qtiles[(0, 1)] = load_q(0, 1)
                        kvtiles[1] = load_kv(1, first=False)[:2]
                        for pqb, pc in ((1, 0), (1, 1), (2, 0), (2, 1)):
                            qtiles[(pqb, pc)] = load_q(pqb, pc)
                    if g == POLYG:
                        ex = expool.tile([128, 3, QCHUNK], BF16, name="ex0", bufs=2)
                        # tile 0 of this group is computed by DVE (poly); ACT
                        # does the other two, shedding one tile per chunk
                        nc.scalar.activation(out=ex[:, 1:3, :],
                                             in_=ps[:, 1:3, 0:QCHUNK],
                                             func=EXP, scale=SCALE)
                        poly_exp(ps[:, 0, 0:QCHUNK], ex[:, 0, :],
                                 after_copy=stash.pop() if stash else None)
                    else:
                        ex = expool.tile([128, 3, QCHUNK], BF16, name="ex")
                        nc.scalar.activation(out=ex, in_=ps[:, :, 0:QCHUNK],
                                             func=EXP, scale=SCALE)
                    exv = ex.rearrange("p a b -> p (a b)")
                    if g == POLYG:
                        gpex, gpexv = ex, exv
                    elif g == (1 if POLYG == 0 else 0):
                        firstexv = exv
                    elif g == 2:
                        nc.vector.tensor_add(acc, firstexv, exv)
                    elif g == NGROUPS - 1:
                        if (si, c) == (len(SCHED) - 1, 1):
                            # last chunk: ship this group raw (host adds it);
                            # the tail then never waits on a final DVE add
                            last_exv = exv
                        else:
                            # stash the final add; emitted after the next
                            # chunk's ps-freeing poly copy
                            def _fin(acc=acc, exv=exv):
                                nc.vector.tensor_add(acc, acc, exv)
                            stash.append(_fin)
                    else:
                        nc.vector.tensor_add(acc, acc, exv)
                        if g == 9:
                            # poly group's contribution, ready by now
                            nc.vector.tensor_add(acc, acc, gpexv)
                    if g == 10:
                        # deferred PV for the poly group (waits on the chain)
                        drain((gpex, VT, po, POLYG, None, False))
                    if len(pend) >= 2:
                        drain(pend.pop(0))
                    epi = None
                    if g == NGROUPS - 1:
                        # chunk epilogue, emitted only after this group's PV
                        # drains so the po copy orders after the last matmul
                        def epi(si=si, c=c, acc=acc, po=po, ex=ex):
                            nc.sync.dma_start(out=dens[si, c], in_=acc)
                            if (si, c) == (len(SCHED) - 1, 1):
                                nc.sync.dma_start(
                                    out=dens2, in_=ex.rearrange("p a b -> p (a b)"))
                            osb = outsb.tile([128, QCHUNK], F32, name="osb")
                            nc.vector.tensor_copy(out=osb, in_=po)
                            nc.sync.dma_start(out=outp[si, c], in_=osb)
                    if g != POLYG:
                        pend.append((ex, VT, po, g, epi,
                                     g == (1 if POLYG == 0 else 0)))
        if stash:
            stash.pop()()
        for p in pend:
            drain(p)

    nc.compile()
    return nc


def _get_program():
    global _PROG
    if _PROG is None:
        _PROG = _build_program()
    return _PROG


def _host_prep(q, k, v, cache_k, cache_v):
    """Build the 8 per-core input maps (rope + layout on host)."""
    import ml_dtypes
    cos, sin = _rope_tables()
    q = np.asarray(q, np.float32)[0]
    k = np.asarray(k, np.float32)[0]
    v = np.asarray(v, np.float32)[0]
    ck = np.asarray(cache_k, np.float32)[0, 1920:5760]
    cv = np.asarray(cache_v, np.float32)[0, 1920:5760]
    rq = _rope_host(q, cos, sin)                     # [1920, 12, 128]
    rk = _rope_host(k, cos, sin)
    Kfull = np.concatenate([ck, rk], axis=0)         # [5760, 12, 128]
    Vfull = np.concatenate([cv, v], axis=0)

    in_maps = []
    for cidx in range(N_CORES):
        dbl, dbl_halves, sgl, sgl_half = _core_heads(cidx)
        qT = np.stack([
            np.ascontiguousarray(rq[dbl_halves[0] * UQ:(dbl_halves[0] + 1) * UQ, dbl, :].T),
            np.ascontiguousarray(rq[dbl_halves[1] * UQ:(dbl_halves[1] + 1) * UQ, dbl, :].T),
            np.ascontiguousarray(rq[sgl_half * UQ:(sgl_half + 1) * UQ, sgl, :].T),
        ])
        kT = np.stack([np.ascontiguousarray(Kfull[:, n, :].T) for n in (dbl, sgl)])
        vT = np.stack([
            np.ascontiguousarray(
                Vfull[:, n, :].reshape(KTILES, 128, 128).transpose(1, 0, 2).reshape(128, WIN))
            for n in (dbl, sgl)
        ]).astype(ml_dtypes.bfloat16)
        in_maps.append({"qT": qT, "kT": kT, "vT": vT})
    return in_maps


def _gather(results):
    out = np.empty((1, S, NHEADS, D), np.float32)
    for cidx in range(N_CORES):
        dbl, dbl_halves, sgl, sgl_half = _core_heads(cidx)
        slots = [(dbl, dbl_halves[0]), (dbl, dbl_halves[1]), (sgl, sgl_half)]
        po = np.asarray(results[cidx]["outp"], np.float32)    # [3, 2, 128, 480]
        de = np.asarray(results[cidx]["dens"], np.float32)    # [3, 2, 128, 1440]
        for s_i, (n, half) in enumerate(slots):
            for c in range(2):
                denom = de[s_i, c].reshape(128, 3, QCHUNK).sum(axis=(0, 1))
                if (s_i, c) == (2, 1):
                    denom = denom + np.asarray(results[cidx]["dens2"], np.float32)\
                        .reshape(128, 3, QCHUNK).sum(axis=(0, 1))
                o = po[s_i, c] / denom[None, :]                # [128, 480]
                q0 = half * UQ + c * QCHUNK
                out[0, q0:q0 + QCHUNK, n, :] = o.T
    return out


def kernel(q, k, v, cache_k, cache_v, f=2, h=24, w=40,
           current_start=5760, global_end=5760, local_end=5760, **_extra):
    from concourse.bass_utils import run_bass_kernel_spmd

    nc = _get_program()
    in_maps = _host_prep(q, k, v, cache_k, cache_v)
    res = run_bass_kernel_spmd(nc, in_maps, list(range(N_CORES)))
    return _gather(res.results)
